# revision 1
# baseline (speedup 1.0000x reference)
"""DySkillHGNN Trainium2 kernel: 6 timesteps x (GAT_p + GAT_c + SAGE) over 30000 nodes.

Sharding: nodes split into 8 ranges of 3750 (one per NeuronCore). Edges are
partitioned by destination owner, sorted into 128-node dst blocks, padded to a
uniform tile count per block. Each core computes out[t, local_nodes, :] for all
t; host reassembles and gathers the s rows.
"""

import numpy as np

import concourse.bass as bass
import concourse.tile as tile
from concourse import bacc, mybir
import concourse.bass_utils as bass_utils
from concourse.masks import make_identity

P = 128
DIM = 128
CORES = 8
HW = 192                          # H table width: [h(128) | a_s | pad]
AW = 64                           # A_D table width: [a_d | pad]
NEG = 0.2
F32 = mybir.dt.float32
I16 = mybir.dt.int16

N_NODES = 30000
N_T = 6
RANGE = N_NODES // CORES          # 3750 nodes per core
BLOCKS = (RANGE + P - 1) // P     # 30 dst blocks per core
NPAD = ((N_NODES + 1 + P - 1) // P) * P   # 30080 table rows
PADROW = N_NODES                  # index of the all-zero row


# ---------------------------------------------------------------- host prep

def _prep_rel(src, dst, selfloop, T):
    """Bucket one relation's edges (all cores) by (core, dst block).

    Returns (src_pad, dst_pad, dloc_pad) of shape [CORES, BLOCKS, T*P] with
    PADROW / -1 padding. T is tiles per block (uniform).
    """
    if selfloop:
        loops = np.arange(N_NODES, dtype=np.int64)
        src = np.concatenate([src, loops])
        dst = np.concatenate([dst, loops])
    key = (dst // RANGE) * BLOCKS + (dst % RANGE) // P      # 0..239
    order = np.argsort(key, kind="stable")
    ks = key[order]
    cnt = np.bincount(ks, minlength=CORES * BLOCKS)
    starts = np.concatenate([[0], np.cumsum(cnt)[:-1]])
    pos = np.arange(len(ks)) - np.repeat(starts, cnt)
    EB = T * P
    assert cnt.max() <= EB, (cnt.max(), EB)
    src_pad = np.full((CORES * BLOCKS, EB), PADROW, np.int64)
    dst_pad = np.full((CORES * BLOCKS, EB), PADROW, np.int64)
    dloc_pad = np.full((CORES * BLOCKS, EB), -1.0, np.float32)
    src_pad[ks, pos] = src[order]
    dst_pad[ks, pos] = dst[order]
    dloc_pad[ks, pos] = (dst[order] % RANGE) % P
    sh = (CORES, BLOCKS, EB)
    return src_pad.reshape(sh), dst_pad.reshape(sh), dloc_pad.reshape(sh)


def _tiles_needed(src, dst, selfloop):
    if selfloop:
        dst = np.concatenate([dst, np.arange(N_NODES, dtype=np.int64)])
    key = (dst // RANGE) * BLOCKS + (dst % RANGE) // P
    return int(-(-np.bincount(key, minlength=CORES * BLOCKS).max() // P))


def _idx_image(a):
    """[..., E_b] int -> DMA-ready int16 image [..., 128, E_b//16]."""
    S = a.shape[-1] // 16
    w = a.reshape(*a.shape[:-1], S, 16)                     # [..., S, 16]
    w = np.swapaxes(w, -1, -2)                              # [..., 16, S]
    w = np.broadcast_to(w[..., None, :, :], (*a.shape[:-1], 8, 16, S))
    return np.ascontiguousarray(w.reshape(*a.shape[:-1], 128, S)).astype(np.int16)


def _dloc_image(a, T):
    """[..., T*P] f32 -> [..., 128, T] (partition p holds edge tile*128+p)."""
    w = a.reshape(*a.shape[:-1], T, P)
    return np.ascontiguousarray(np.swapaxes(w, -1, -2))


# ---------------------------------------------------------------- device program

def build_program(T_G, T_S, repeats=1, mode="full"):
    """Build the SPMD Bass program. T_G / T_S: tiles per GAT / SAGE block."""
    S_G, S_S = T_G * 8, T_S * 8          # idx image cols (= T*128/16)
    NB = N_T * BLOCKS                    # 180 (t, block) iterations

    nc = bacc.Bacc("TRN2", target_bir_lowering=False, debug=False,
                   num_devices=CORES, num_swdge_queues=4)

    def din(name, shape, dt=F32):
        return nc.dram_tensor(name, shape, dt, kind="ExternalInput")

    x_pad = din("x_pad", [NPAD, DIM])            # row PADROW.. zeros
    x_t = din("x_t", [P, NPAD])                  # x transposed (host)
    w_p = din("w_p", [DIM, DIM])
    w_c = din("w_c", [DIM, DIM])
    w_l = din("w_l", [DIM, DIM])
    wr3 = din("wr3", [DIM, DIM])                 # W_r / 3
    att = din("att", [DIM, 4])                   # [src_p | dst_p | src_c | dst_c]
    bias = din("bias", [1, DIM])                 # (b_p+b_c+b_l+b_r)/3
    sidx_p = din("sidx_p", [NB * P, S_G], I16)
    didx_p = din("didx_p", [NB * P, S_G], I16)
    sidx_c = din("sidx_c", [NB * P, S_G], I16)
    didx_c = din("didx_c", [NB * P, S_G], I16)
    sidx_s = din("sidx_s", [NB * P, S_S], I16)
    dloc_p = din("dloc_p", [NB * P, T_G])
    dloc_c = din("dloc_c", [NB * P, T_G])
    dloc_s = din("dloc_s", [NB * P, T_S])
    xr3_l = din("xr3_l", [BLOCKS * P, DIM])      # core-local x@(W_r/3) rows

    out_d = nc.dram_tensor("out", [NB * P, DIM], F32, kind="ExternalOutput")

    ht_p = nc.dram_tensor("ht_p", [NPAD, HW], F32, kind="Internal")
    ht_c = nc.dram_tensor("ht_c", [NPAD, HW], F32, kind="Internal")
    ad_p = nc.dram_tensor("ad_p", [NPAD, AW], F32, kind="Internal")
    ad_c = nc.dram_tensor("ad_c", [NPAD, AW], F32, kind="Internal")

    with tile.TileContext(nc) as tc:
        with tc.tile_pool(name="const", bufs=1) as cpool:
            ident = cpool.tile([P, P], F32)
            make_identity(nc, ident[:])
            iota_f = cpool.tile([P, P], F32)
            nc.gpsimd.iota(iota_f[:], pattern=[[1, P]], base=0,
                           channel_multiplier=0,
                           allow_small_or_imprecise_dtypes=True)
            iota4 = cpool.tile([P, 4, P], F32)
            nc.gpsimd.iota(iota4[:], pattern=[[0, 4], [1, P]], base=0,
                           channel_multiplier=0,
                           allow_small_or_imprecise_dtypes=True)
            ones_col = cpool.tile([P, 1], F32)
            nc.vector.memset(ones_col[:], 1.0)
            wl_t = cpool.tile([DIM, DIM], F32)
            nc.sync.dma_start(wl_t[:], w_l[:])
            bias_bc = cpool.tile([P, DIM], F32)
            bias_row = cpool.tile([1, DIM], F32)
            nc.sync.dma_start(bias_row[:], bias[:])
            nc.gpsimd.partition_broadcast(bias_bc[:], bias_row[:])

            # ---------------- prologue: build H/A_D/X tables ----------------
            with tc.tile_pool(name="prol", bufs=3) as pp, \
                 tc.tile_pool(name="prolp", bufs=2, space="PSUM") as ppp, \
                 tc.tile_pool(name="xtp", bufs=2) as xtp:
                wp_t = pp.tile([DIM, DIM], F32, tag="wp")
                nc.sync.dma_start(wp_t[:], w_p[:])
                wc_t = pp.tile([DIM, DIM], F32, tag="wc")
                nc.sync.dma_start(wc_t[:], w_c[:])
                wr3_t = pp.tile([DIM, DIM], F32, tag="wr")
                nc.sync.dma_start(wr3_t[:], wr3[:])
                att_t = pp.tile([DIM, 4], F32, tag="att")
                nc.sync.dma_start(att_t[:], att[:])
                # v[dim, 4] = W @ att ( = x-side projection of att vectors )
                # a_s = x @ (W_p @ att_src_p): lhsT = x_T, rhs = v columns
                wpT_ps = ppp.tile([DIM, DIM], F32, tag="wT")
                nc.tensor.transpose(wpT_ps[:], wp_t[:], ident[:])
                wpT = pp.tile([DIM, DIM], F32, tag="wpT")
                nc.scalar.copy(wpT[:], wpT_ps[:])
                wcT_ps = ppp.tile([DIM, DIM], F32, tag="wT")
                nc.tensor.transpose(wcT_ps[:], wc_t[:], ident[:])
                wcT = pp.tile([DIM, DIM], F32, tag="wcT")
                nc.scalar.copy(wcT[:], wcT_ps[:])
                v_ps = ppp.tile([DIM, 4], F32, tag="v")
                nc.tensor.matmul(v_ps[:, 0:2], lhsT=wpT[:], rhs=att_t[:, 0:2],
                                 start=True, stop=True)
                nc.tensor.matmul(v_ps[:, 2:4], lhsT=wcT[:], rhs=att_t[:, 2:4],
                                 start=True, stop=True)
                v_t = pp.tile([DIM, 4], F32, tag="vt")
                nc.scalar.copy(v_t[:], v_ps[:])

                NQ = 5
                QCH = (NPAD // P + NQ - 1) // NQ      # chunks per quarter-load
                for q in range(NQ):
                  c_lo = q * QCH
                  c_hi = min((q + 1) * QCH, NPAD // P)
                  xts = xtp.tile([P, QCH * P], F32, tag="xts")
                  nc.sync.dma_start(xts[:, 0:(c_hi - c_lo) * P],
                                    x_t[:, c_lo * P:c_hi * P])
                  for c in range(c_lo, c_hi):
                    sl = slice(c * P, (c + 1) * P)
                    xt_c = xts[:, (c - c_lo) * P:(c - c_lo + 1) * P]
                    for (wt, htab, adtab, vs, vd) in (
                        (wp_t, ht_p, ad_p, 0, 1),
                        (wc_t, ht_c, ad_c, 2, 3),
                    ):
                        h_ps = ppp.tile([P, DIM], F32, tag="h")
                        nc.tensor.matmul(h_ps[:], lhsT=xt_c, rhs=wt[:],
                                         start=True, stop=True)
                        a_ps = ppp.tile([P, 2], F32, tag="a")
                        nc.tensor.matmul(a_ps[:], lhsT=xt_c, rhs=v_t[:, vs:vd + 1],
                                         start=True, stop=True)
                        htile = pp.tile([P, HW], F32, tag="htile")
                        nc.scalar.copy(htile[:, 0:DIM], h_ps[:])
                        nc.scalar.copy(htile[:, DIM:DIM + 1], a_ps[:, 0:1])
                        nc.vector.memset(htile[:, DIM + 1:], 0.0)
                        nc.sync.dma_start(htab[sl, :], htile[:])
                        atile = pp.tile([P, AW], F32, tag="atile")
                        nc.scalar.copy(atile[:, 0:1], a_ps[:, 1:2])
                        nc.vector.memset(atile[:, 1:], 0.0)
                        nc.sync.dma_start(adtab[sl, :], atile[:])

            # ---------------- edge phase ----------------
            with tc.tile_pool(name="meta", bufs=3) as mp, \
                 tc.tile_pool(name="gath", bufs=3) as gp, \
                 tc.tile_pool(name="wrk", bufs=3) as wp_pool, \
                 tc.tile_pool(name="omat", bufs=3) as op, \
                 tc.tile_pool(name="evac", bufs=3) as ep, \
                 tc.tile_pool(name="eps", bufs=1, space="PSUM") as eps, \
                 tc.tile_pool(name="tps", bufs=2, space="PSUM") as tps:

                for _rep in range(repeats):
                  with tc.For_i(0, BLOCKS, 1) as blk:
                      for t in range(N_T):
                          row0 = blk * P + t * (BLOCKS * P)
                          rows = bass.ds(row0, P)
                          num_p = eps.tile([DIM, P], F32, tag="num_p")
                          num_c = eps.tile([DIM, P], F32, tag="num_c")
                          num_s = eps.tile([DIM, P], F32, tag="num_s")
                          dens = eps.tile([1, 3 * P], F32, tag="dens")

                          # ---- pass A: issue all index DMAs + gathers
                          rels = (
                              (sidx_p, didx_p, dloc_p, ht_p, T_G, S_G, True),
                              (sidx_c, didx_c, dloc_c, ht_c, T_G, S_G, True),
                              (sidx_s, None, dloc_s, x_pad, T_S, S_S, False),
                          )
                          gathered = []
                          for (ri, (sidx, didx, dloc, htab, T_R, S_R, gat)) in enumerate(rels):
                              WROW = HW if gat else DIM
                              si_t = mp.tile([P, S_R], I16, tag=f"si{ri}")
                              nc.sync.dma_start(si_t[:], sidx[rows, :])
                              dl_t = mp.tile([P, T_R], F32, tag=f"dl{ri}")
                              nc.sync.dma_start(dl_t[:], dloc[rows, :])
                              hg = gp.tile([P, T_R, WROW], F32, tag=f"hg{ri}")
                              nc.gpsimd.dma_gather(hg[:], htab[:], si_t[:],
                                                   T_R * P, T_R * P, WROW,
                                                   single_packet=False,
                                                   queue_num=(t * 5 + ri * 2) % 4)
                              ag = None
                              if gat:
                                  di_t = mp.tile([P, S_R], I16, tag=f"di{ri}")
                                  nc.sync.dma_start(di_t[:], didx[rows, :])
                                  adtab = ad_p if ri == 0 else ad_c
                                  ag = gp.tile([P, T_R, AW], F32, tag=f"ag{ri}")
                                  nc.gpsimd.dma_gather(ag[:], adtab[:], di_t[:],
                                                       T_R * P, T_R * P, AW,
                                                       single_packet=False,
                                                       queue_num=(t * 5 + ri * 2 + 1) % 4)
                              gathered.append((hg, ag, dl_t))
                          # ---- pass B: compute per relation
                          for (ri, (sidx, didx, dloc, htab, T_R, S_R, gat)) in enumerate(rels):
                              num_ps = (num_p, num_c, num_s)[ri]
                              hg, ag, dl_t = gathered[ri]
                              if gat:
                                  if mode == "gather":
                                      continue
                                  s_t = wp_pool.tile([P, T_R], F32, tag=f"s{ri}")
                                  nc.vector.tensor_tensor(
                                      out=s_t[:], in0=hg[:, :, DIM],
                                      in1=ag[:, :, 0], op=mybir.AluOpType.add)
                                  lr_t = wp_pool.tile([P, T_R], F32, tag=f"lr{ri}")
                                  nc.scalar.activation(
                                      lr_t[:], s_t[:],
                                      mybir.ActivationFunctionType.Prelu, alpha=NEG)
                                  w_t = wp_pool.tile([P, T_R], F32, tag=f"w{ri}")
                                  nc.scalar.activation(
                                      w_t[:], lr_t[:],
                                      mybir.ActivationFunctionType.Exp)
                              if mode == "gather":
                                  continue
                              for g0 in range(0, T_R, 4):
                                  gl = min(4, T_R - g0)
                                  o4 = op.tile([P, 4, P], F32, tag=f"or{ri}")
                                  nc.vector.tensor_tensor(
                                      out=o4[:, 0:gl, :], in0=iota4[:, 0:gl, :],
                                      in1=dl_t[:, g0:g0 + gl].unsqueeze(2)
                                          .to_broadcast([P, gl, P]),
                                      op=mybir.AluOpType.is_equal)
                                  if gat:
                                      ow4 = op.tile([P, 4, P], F32, tag=f"ow{ri}")
                                      nc.vector.tensor_tensor(
                                          out=ow4[:, 0:gl, :], in0=o4[:, 0:gl, :],
                                          in1=w_t[:, g0:g0 + gl].unsqueeze(2)
                                              .to_broadcast([P, gl, P]),
                                          op=mybir.AluOpType.mult)
                                      o_mm = ow4
                                  else:
                                      o_mm = o4
                                  for j in range(gl):
                                      k = g0 + j
                                      nc.tensor.matmul(num_ps[:],
                                                       lhsT=hg[:, k, 0:DIM],
                                                       rhs=o_mm[:, j, :],
                                                       start=(k == 0),
                                                       stop=(k == T_R - 1))
                                  ng = (T_R + 3) // 4
                                  gi = g0 // 4
                                  if gl == 1:
                                      osum = o_mm[:, 0, :]
                                  else:
                                      ost = op.tile([P, P], F32, tag=f"os{ri}")
                                      nc.vector.tensor_reduce(
                                          out=ost[:],
                                          in_=o_mm[:, 0:gl, :].rearrange(
                                              "p g d -> p d g"),
                                          axis=mybir.AxisListType.X,
                                          op=mybir.AluOpType.add)
                                      osum = ost[:]
                                  nc.tensor.matmul(dens[:, ri * P:(ri + 1) * P],
                                                   lhsT=ones_col[:], rhs=osum,
                                                   start=(gi == 0),
                                                   stop=(gi == ng - 1))

                          # ---- normalize + combine
                          if mode != "full":
                              ot = ep.tile([P, DIM], F32, tag="out_t")
                              if mode == "gather":
                                  nc.vector.memset(ot[:], 0.0)
                              else:
                                  nc.vector.tensor_copy(ot[:], num_p[:])
                              nc.sync.dma_start(out_d[rows, :], ot[:])
                              continue
                          dens_s = ep.tile([1, 3 * P], F32, tag="dens_s")
                          nc.vector.tensor_copy(dens_s[:], dens[:])
                          dcol = tps.tile([P, 3], F32, tag="dcol")
                          for ri in range(3):
                              nc.tensor.transpose(dcol[:, ri:ri + 1],
                                                  dens_s[:, ri * P:(ri + 1) * P],
                                                  ident[:1, :1])
                          r_p = ep.tile([P, 1], F32, tag="r_p")
                          nc.vector.tensor_scalar(out=r_p[:], in0=dcol[:, 0:1],
                                                  scalar1=3.0, scalar2=None,
                                                  op0=mybir.AluOpType.mult)
                          nc.vector.reciprocal(r_p[:], r_p[:])
                          r_c = ep.tile([P, 1], F32, tag="r_c")
                          nc.vector.tensor_scalar(out=r_c[:], in0=dcol[:, 1:2],
                                                  scalar1=3.0, scalar2=None,
                                                  op0=mybir.AluOpType.mult)
                          nc.vector.reciprocal(r_c[:], r_c[:])
                          r_s = ep.tile([P, 1], F32, tag="r_s")
                          nc.vector.tensor_scalar(out=r_s[:], in0=dcol[:, 2:3],
                                                  scalar1=1.0, scalar2=3.0,
                                                  op0=mybir.AluOpType.max,
                                                  op1=mybir.AluOpType.mult)
                          nc.vector.reciprocal(r_s[:], r_s[:])

                          acc = ep.tile([P, DIM], F32, tag="acc")
                          numT = ep.tile([DIM, P], F32, tag="numT")
                          nc.scalar.copy(numT[:], num_p[:])
                          numn = tps.tile([P, DIM], F32, tag="numn")
                          nc.tensor.transpose(numn[:], numT[:], ident[:])
                          nc.vector.tensor_scalar(out=acc[:], in0=numn[:],
                                                  scalar1=r_p[:], scalar2=None,
                                                  op0=mybir.AluOpType.mult)
                          numT2 = ep.tile([DIM, P], F32, tag="numT2")
                          nc.scalar.copy(numT2[:], num_c[:])
                          numn2 = tps.tile([P, DIM], F32, tag="numn")
                          nc.tensor.transpose(numn2[:], numT2[:], ident[:])
                          tmp = ep.tile([P, DIM], F32, tag="tmp")
                          nc.vector.tensor_scalar(out=tmp[:], in0=numn2[:],
                                                  scalar1=r_c[:], scalar2=None,
                                                  op0=mybir.AluOpType.mult)
                          nc.vector.tensor_add(acc[:], acc[:], tmp[:])
                          # SAGE: (num_s @ W_l) scaled
                          numT3 = ep.tile([DIM, P], F32, tag="numT3")
                          nc.scalar.copy(numT3[:], num_s[:])
                          z_ps = tps.tile([DIM, P], F32, tag="numn")
                          nc.tensor.matmul(z_ps[:], lhsT=wl_t[:], rhs=numT3[:],
                                           start=True, stop=True)
                          z_s = ep.tile([DIM, P], F32, tag="z_s")
                          nc.scalar.copy(z_s[:], z_ps[:])
                          zn = tps.tile([P, DIM], F32, tag="numn")
                          nc.tensor.transpose(zn[:], z_s[:], ident[:])
                          tmp2 = ep.tile([P, DIM], F32, tag="tmp2")
                          nc.vector.tensor_scalar(out=tmp2[:], in0=zn[:],
                                                  scalar1=r_s[:], scalar2=None,
                                                  op0=mybir.AluOpType.mult)
                          nc.vector.tensor_add(acc[:], acc[:], tmp2[:])
                          xr_t = ep.tile([P, DIM], F32, tag="xr_t")
                          nc.sync.dma_start(xr_t[:], xr3_l[bass.ds(blk * P, P), :])
                          nc.vector.tensor_add(acc[:], acc[:], xr_t[:])
                          out_t = ep.tile([P, DIM], F32, tag="out_t")
                          nc.vector.tensor_add(out_t[:], acc[:], bias_bc[:])
                          nc.sync.dma_start(out_d[rows, :], out_t[:])

    nc.compile()
    return nc


# ---------------------------------------------------------------- entry

def _run(inputs, trace=False):
    s = np.asarray(inputs["s"])
    edge_index = np.asarray(inputs["edge_index"])
    x = np.asarray(inputs["embed_weight"], dtype=np.float32)
    W_p = np.asarray(inputs["W_p"], np.float32)
    W_c = np.asarray(inputs["W_c"], np.float32)
    W_l = np.asarray(inputs["W_l"], np.float32)
    W_r = np.asarray(inputs["W_r"], np.float32)
    att = np.stack([np.asarray(inputs["att_src_p"], np.float32),
                    np.asarray(inputs["att_dst_p"], np.float32),
                    np.asarray(inputs["att_src_c"], np.float32),
                    np.asarray(inputs["att_dst_c"], np.float32)], axis=1)
    bias = ((np.asarray(inputs["b_p"], np.float32)
             + np.asarray(inputs["b_c"], np.float32)
             + np.asarray(inputs["b_l"], np.float32)
             + np.asarray(inputs["b_r"], np.float32)) / 3.0)[None, :]

    # tiles needed (uniform across t / cores per relation kind)
    T_G = max(
        max(_tiles_needed(edge_index[t, r, 0], edge_index[t, r, 1], True)
            for r in (0, 1))
        for t in range(N_T))
    T_S = max(_tiles_needed(edge_index[t, 2, 0], edge_index[t, 2, 1], False)
              for t in range(N_T))

    # host-side edge bucketing; arrays [N_T, CORES, BLOCKS, ...]
    rel_data = {}
    for name, r, loop, T in (("p", 0, True, T_G), ("c", 1, True, T_G),
                             ("s", 2, False, T_S)):
        sp = np.empty((N_T, CORES, BLOCKS, T * P), np.int64)
        dp = np.empty_like(sp)
        dl = np.empty((N_T, CORES, BLOCKS, T * P), np.float32)
        for t in range(N_T):
            sp[t], dp[t], dl[t] = _prep_rel(edge_index[t, r, 0],
                                            edge_index[t, r, 1], loop, T)
        rel_data[name] = (sp, dp, dl, T)

    x_pad = np.zeros((NPAD, DIM), np.float32)
    x_pad[:N_NODES] = x
    x_t = np.ascontiguousarray(x_pad.T)
    xr3 = x_pad @ (W_r / 3.0)

    nc = build_program(T_G, T_S)

    in_maps = []
    for core in range(CORES):
        m = {
            "x_pad": x_pad, "x_t": x_t, "w_p": W_p, "w_c": W_c, "w_l": W_l,
            "wr3": (W_r / 3.0).astype(np.float32), "att": att, "bias": bias,
            "xr3_l": np.concatenate(
                [xr3[core * RANGE:(core + 1) * RANGE],
                 np.zeros((BLOCKS * P - RANGE, DIM), np.float32)]),
        }
        for name in ("p", "c", "s"):
            sp, dp, dl, T = rel_data[name]
            S = T * 8
            img = _idx_image(sp[:, core])            # [N_T, BLOCKS, 128, S]
            m[f"sidx_{name}"] = img.reshape(-1, S)
            if name != "s":
                m[f"didx_{name}"] = _idx_image(dp[:, core]).reshape(-1, S)
            m[f"dloc_{name}"] = _dloc_image(dl[:, core], T).reshape(-1, T)
        in_maps.append(m)

    res = bass_utils.run_bass_kernel_spmd(nc, in_maps,
                                          core_ids=list(range(CORES)),
                                          trace=trace)
    # assemble: per core out [N_T*BLOCKS*P, DIM] -> [N_T, BLOCKS*P, DIM]
    full = np.empty((N_T, N_NODES, DIM), np.float32)
    for core in range(CORES):
        o = res.results[core]["out"].reshape(N_T, BLOCKS * P, DIM)
        full[:, core * RANGE:(core + 1) * RANGE] = o[:, :RANGE]
    return full[:, s, :], res


def kernel(**inputs):
    """Full (unsharded) inputs -> full output [N_T, len(s), DIM] float32."""
    out, _ = _run(inputs)
    return out



# revision 2
# speedup vs baseline: 254.1042x; 254.1042x over previous
"""DySkillHGNN Trainium2 kernel: 6 timesteps x (GAT_p + GAT_c + SAGE) over 30000 nodes.

Output sparsity: the model only returns rows s (~1015 unique node ids) of each
timestep, so only edges whose destination is in unique(s) matter. Slots
g = t*U + slot(dst) (~6090 of them) are packed into 48 blocks of 128 and
sharded 6-per-core across 8 NeuronCores. Edges are bucketed by slot block,
padded to a uniform tile count. Each core computes out rows for its 6 blocks;
host reassembles and expands to [6, |s|, D].
"""

import numpy as np

import concourse.bass as bass
import concourse.tile as tile
from concourse import bacc, mybir
import concourse.bass_utils as bass_utils
from concourse.masks import make_identity

P = 128
DIM = 128
CORES = 8
HW = 192                          # H table width: [h(128) | a_s | pad]
AW = 64                           # A_D table width: [a_d | pad]
NEG = 0.2
F32 = mybir.dt.float32
I16 = mybir.dt.int16

N_NODES = 30000
N_T = 6
NPAD = ((N_NODES + 1 + P - 1) // P) * P   # 30080 table rows
PADROW = N_NODES                  # index of the all-zero row

_CB = [6]                         # blocks per core (set by _run before build)


# ---------------------------------------------------------------- host prep

def _prep_rel(edge_index, r, selfloop, uniq, slot_of, U, NBLK, T=None):
    """Bucket one relation's kept edges (all t) by slot block.

    Returns (src_pad, dst_pad, dloc_pad, T) of shape [NBLK, T*P] with
    PADROW / -1 padding.
    """
    gs, srcs = [], []
    for t in range(N_T):
        src = np.asarray(edge_index[t, r, 0])
        dst = np.asarray(edge_index[t, r, 1])
        sl = slot_of[dst]
        keep = sl >= 0
        g = t * U + sl[keep]
        sk = src[keep]
        if selfloop:
            g = np.concatenate([g, t * U + np.arange(U, dtype=np.int64)])
            sk = np.concatenate([sk, uniq])
        gs.append(g)
        srcs.append(sk)
    g = np.concatenate(gs)
    src = np.concatenate(srcs)
    order = np.argsort(g, kind="stable")
    gs_ = g[order]
    src_ = src[order]
    blk = gs_ // P
    cnt = np.bincount(blk, minlength=NBLK)
    if T is None:
        T = int(-(-cnt.max() // P))
    EB = T * P
    assert cnt.max() <= EB, (cnt.max(), EB)
    starts = np.concatenate([[0], np.cumsum(cnt)[:-1]])
    pos = np.arange(len(gs_)) - np.repeat(starts, cnt)
    src_pad = np.full((NBLK, EB), PADROW, np.int64)
    dst_pad = np.full((NBLK, EB), PADROW, np.int64)
    dloc_pad = np.full((NBLK, EB), -1.0, np.float32)
    src_pad[blk, pos] = src_
    dst_pad[blk, pos] = uniq[gs_ % U]
    dloc_pad[blk, pos] = gs_ % P
    return src_pad, dst_pad, dloc_pad, T


def _count_tiles(edge_index, r, selfloop, slot_of, U, NBLK):
    gs = []
    for t in range(N_T):
        dst = np.asarray(edge_index[t, r, 1])
        sl = slot_of[dst]
        g = t * U + sl[sl >= 0]
        gs.append(g)
        if selfloop:
            gs.append(t * U + np.arange(U, dtype=np.int64))
    cnt = np.bincount(np.concatenate(gs) // P, minlength=NBLK)
    return int(-(-cnt.max() // P))


def _idx_image(a):
    """[..., E_b] int -> DMA-ready int16 image [..., 128, E_b//16]."""
    S = a.shape[-1] // 16
    w = a.reshape(*a.shape[:-1], S, 16)                     # [..., S, 16]
    w = np.swapaxes(w, -1, -2)                              # [..., 16, S]
    w = np.broadcast_to(w[..., None, :, :], (*a.shape[:-1], 8, 16, S))
    return np.ascontiguousarray(w.reshape(*a.shape[:-1], 128, S)).astype(np.int16)


def _dloc_image(a, T):
    """[..., T*P] f32 -> [..., 128, T] (partition p holds edge tile*128+p)."""
    w = a.reshape(*a.shape[:-1], T, P)
    return np.ascontiguousarray(np.swapaxes(w, -1, -2))


# ---------------------------------------------------------------- device program

def build_program(T_G, T_S, repeats=1, mode="full"):
    """Build the SPMD Bass program. T_G / T_S: tiles per GAT / SAGE block."""
    S_G, S_S = T_G * 8, T_S * 8          # idx image cols (= T*128/16)
    CB = _CB[0]                          # blocks per core

    nc = bacc.Bacc("TRN2", target_bir_lowering=False, debug=False,
                   num_devices=CORES, num_swdge_queues=4)

    def din(name, shape, dt=F32):
        return nc.dram_tensor(name, shape, dt, kind="ExternalInput")

    x_pad = din("x_pad", [NPAD, DIM])            # row PADROW.. zeros
    x_t = din("x_t", [P, NPAD])                  # x transposed (host)
    w_p = din("w_p", [DIM, DIM])
    w_c = din("w_c", [DIM, DIM])
    w_l = din("w_l", [DIM, DIM])
    att = din("att", [DIM, 4])                   # [src_p | dst_p | src_c | dst_c]
    bias = din("bias", [1, DIM])                 # (b_p+b_c+b_l+b_r)/3
    sidx_p = din("sidx_p", [CB * P, S_G], I16)
    didx_p = din("didx_p", [CB * P, S_G], I16)
    sidx_c = din("sidx_c", [CB * P, S_G], I16)
    didx_c = din("didx_c", [CB * P, S_G], I16)
    sidx_s = din("sidx_s", [CB * P, S_S], I16)
    dloc_p = din("dloc_p", [CB * P, T_G])
    dloc_c = din("dloc_c", [CB * P, T_G])
    dloc_s = din("dloc_s", [CB * P, T_S])
    xr3_l = din("xr3_l", [CB * P, DIM])          # core-local x@(W_r/3) rows

    out_d = nc.dram_tensor("out", [CB * P, DIM], F32, kind="ExternalOutput")

    ht_p = nc.dram_tensor("ht_p", [NPAD, HW], F32, kind="Internal")
    ht_c = nc.dram_tensor("ht_c", [NPAD, HW], F32, kind="Internal")
    ad_p = nc.dram_tensor("ad_p", [NPAD, AW], F32, kind="Internal")
    ad_c = nc.dram_tensor("ad_c", [NPAD, AW], F32, kind="Internal")

    with tile.TileContext(nc) as tc:
        with tc.tile_pool(name="const", bufs=1) as cpool:
            ident = cpool.tile([P, P], F32)
            make_identity(nc, ident[:])
            iota4 = cpool.tile([P, 4, P], F32)
            nc.gpsimd.iota(iota4[:], pattern=[[0, 4], [1, P]], base=0,
                           channel_multiplier=0,
                           allow_small_or_imprecise_dtypes=True)
            ones_col = cpool.tile([P, 1], F32)
            nc.vector.memset(ones_col[:], 1.0)
            wl_t = cpool.tile([DIM, DIM], F32)
            nc.sync.dma_start(wl_t[:], w_l[:])
            bias_bc = cpool.tile([P, DIM], F32)
            bias_row = cpool.tile([1, DIM], F32)
            nc.sync.dma_start(bias_row[:], bias[:])
            nc.gpsimd.partition_broadcast(bias_bc[:], bias_row[:])

            # ---------------- prologue: build H/A_D tables ----------------
            with tc.tile_pool(name="prol", bufs=3) as pp, \
                 tc.tile_pool(name="prolp", bufs=2, space="PSUM") as ppp, \
                 tc.tile_pool(name="xtp", bufs=2) as xtp:
                wp_t = pp.tile([DIM, DIM], F32, tag="wp")
                nc.sync.dma_start(wp_t[:], w_p[:])
                wc_t = pp.tile([DIM, DIM], F32, tag="wc")
                nc.sync.dma_start(wc_t[:], w_c[:])
                att_t = pp.tile([DIM, 4], F32, tag="att")
                nc.sync.dma_start(att_t[:], att[:])
                # v[dim, 4] = W @ att ( = x-side projection of att vectors )
                wpT_ps = ppp.tile([DIM, DIM], F32, tag="wT")
                nc.tensor.transpose(wpT_ps[:], wp_t[:], ident[:])
                wpT = pp.tile([DIM, DIM], F32, tag="wpT")
                nc.scalar.copy(wpT[:], wpT_ps[:])
                wcT_ps = ppp.tile([DIM, DIM], F32, tag="wT")
                nc.tensor.transpose(wcT_ps[:], wc_t[:], ident[:])
                wcT = pp.tile([DIM, DIM], F32, tag="wcT")
                nc.scalar.copy(wcT[:], wcT_ps[:])
                v_ps = ppp.tile([DIM, 4], F32, tag="v")
                nc.tensor.matmul(v_ps[:, 0:2], lhsT=wpT[:], rhs=att_t[:, 0:2],
                                 start=True, stop=True)
                nc.tensor.matmul(v_ps[:, 2:4], lhsT=wcT[:], rhs=att_t[:, 2:4],
                                 start=True, stop=True)
                v_t = pp.tile([DIM, 4], F32, tag="vt")
                nc.scalar.copy(v_t[:], v_ps[:])

                NQ = 5
                QCH = (NPAD // P + NQ - 1) // NQ      # chunks per quarter-load
                for q in range(NQ):
                  c_lo = q * QCH
                  c_hi = min((q + 1) * QCH, NPAD // P)
                  xts = xtp.tile([P, QCH * P], F32, tag="xts")
                  nc.sync.dma_start(xts[:, 0:(c_hi - c_lo) * P],
                                    x_t[:, c_lo * P:c_hi * P])
                  for c in range(c_lo, c_hi):
                    sl = slice(c * P, (c + 1) * P)
                    xt_c = xts[:, (c - c_lo) * P:(c - c_lo + 1) * P]
                    for (wt, htab, adtab, vs, vd) in (
                        (wp_t, ht_p, ad_p, 0, 1),
                        (wc_t, ht_c, ad_c, 2, 3),
                    ):
                        h_ps = ppp.tile([P, DIM], F32, tag="h")
                        nc.tensor.matmul(h_ps[:], lhsT=xt_c, rhs=wt[:],
                                         start=True, stop=True)
                        a_ps = ppp.tile([P, 2], F32, tag="a")
                        nc.tensor.matmul(a_ps[:], lhsT=xt_c, rhs=v_t[:, vs:vd + 1],
                                         start=True, stop=True)
                        htile = pp.tile([P, HW], F32, tag="htile")
                        nc.scalar.copy(htile[:, 0:DIM], h_ps[:])
                        nc.scalar.copy(htile[:, DIM:DIM + 1], a_ps[:, 0:1])
                        nc.vector.memset(htile[:, DIM + 1:], 0.0)
                        nc.sync.dma_start(htab[sl, :], htile[:])
                        atile = pp.tile([P, AW], F32, tag="atile")
                        nc.scalar.copy(atile[:, 0:1], a_ps[:, 1:2])
                        nc.vector.memset(atile[:, 1:], 0.0)
                        nc.sync.dma_start(adtab[sl, :], atile[:])

            # ---------------- edge phase ----------------
            with tc.tile_pool(name="meta", bufs=3) as mp, \
                 tc.tile_pool(name="gath", bufs=3) as gp, \
                 tc.tile_pool(name="wrk", bufs=3) as wp_pool, \
                 tc.tile_pool(name="omat", bufs=3) as op, \
                 tc.tile_pool(name="evac", bufs=3) as ep, \
                 tc.tile_pool(name="eps", bufs=1, space="PSUM") as eps, \
                 tc.tile_pool(name="tps", bufs=2, space="PSUM") as tps:

                for _rep in range(repeats):
                  for blk in range(CB):
                      row0 = blk * P
                      rows = bass.ds(row0, P)
                      num_p = eps.tile([DIM, P], F32, tag="num_p")
                      num_c = eps.tile([DIM, P], F32, tag="num_c")
                      num_s = eps.tile([DIM, P], F32, tag="num_s")
                      dens = eps.tile([1, 3 * P], F32, tag="dens")

                      # ---- pass A: issue all index DMAs + gathers
                      rels = (
                          (sidx_p, didx_p, dloc_p, ht_p, T_G, S_G, True),
                          (sidx_c, didx_c, dloc_c, ht_c, T_G, S_G, True),
                          (sidx_s, None, dloc_s, x_pad, T_S, S_S, False),
                      )
                      gathered = []
                      for (ri, (sidx, didx, dloc, htab, T_R, S_R, gat)) in enumerate(rels):
                          WROW = HW if gat else DIM
                          si_t = mp.tile([P, S_R], I16, tag=f"si{ri}")
                          nc.sync.dma_start(si_t[:], sidx[rows, :])
                          dl_t = mp.tile([P, T_R], F32, tag=f"dl{ri}")
                          nc.sync.dma_start(dl_t[:], dloc[rows, :])
                          hg = gp.tile([P, T_R, WROW], F32, tag=f"hg{ri}")
                          nc.gpsimd.dma_gather(hg[:], htab[:], si_t[:],
                                               T_R * P, T_R * P, WROW,
                                               single_packet=False,
                                               queue_num=(blk * 5 + ri * 2) % 4)
                          ag = None
                          if gat:
                              di_t = mp.tile([P, S_R], I16, tag=f"di{ri}")
                              nc.sync.dma_start(di_t[:], didx[rows, :])
                              adtab = ad_p if ri == 0 else ad_c
                              ag = gp.tile([P, T_R, AW], F32, tag=f"ag{ri}")
                              nc.gpsimd.dma_gather(ag[:], adtab[:], di_t[:],
                                                   T_R * P, T_R * P, AW,
                                                   single_packet=False,
                                                   queue_num=(blk * 5 + ri * 2 + 1) % 4)
                          gathered.append((hg, ag, dl_t))
                      # ---- pass B: compute per relation
                      for (ri, (sidx, didx, dloc, htab, T_R, S_R, gat)) in enumerate(rels):
                          num_ps = (num_p, num_c, num_s)[ri]
                          hg, ag, dl_t = gathered[ri]
                          if gat:
                              if mode == "gather":
                                  continue
                              s_t = wp_pool.tile([P, T_R], F32, tag=f"s{ri}")
                              nc.vector.tensor_tensor(
                                  out=s_t[:], in0=hg[:, :, DIM],
                                  in1=ag[:, :, 0], op=mybir.AluOpType.add)
                              lr_t = wp_pool.tile([P, T_R], F32, tag=f"lr{ri}")
                              nc.scalar.activation(
                                  lr_t[:], s_t[:],
                                  mybir.ActivationFunctionType.Prelu, alpha=NEG)
                              w_t = wp_pool.tile([P, T_R], F32, tag=f"w{ri}")
                              nc.scalar.activation(
                                  w_t[:], lr_t[:],
                                  mybir.ActivationFunctionType.Exp)
                          if mode == "gather":
                              continue
                          for g0 in range(0, T_R, 4):
                              gl = min(4, T_R - g0)
                              o4 = op.tile([P, 4, P], F32, tag=f"or{ri}")
                              nc.vector.tensor_tensor(
                                  out=o4[:, 0:gl, :], in0=iota4[:, 0:gl, :],
                                  in1=dl_t[:, g0:g0 + gl].unsqueeze(2)
                                      .to_broadcast([P, gl, P]),
                                  op=mybir.AluOpType.is_equal)
                              if gat:
                                  ow4 = op.tile([P, 4, P], F32, tag=f"ow{ri}")
                                  nc.vector.tensor_tensor(
                                      out=ow4[:, 0:gl, :], in0=o4[:, 0:gl, :],
                                      in1=w_t[:, g0:g0 + gl].unsqueeze(2)
                                          .to_broadcast([P, gl, P]),
                                      op=mybir.AluOpType.mult)
                                  o_mm = ow4
                              else:
                                  o_mm = o4
                              for j in range(gl):
                                  k = g0 + j
                                  nc.tensor.matmul(num_ps[:],
                                                   lhsT=hg[:, k, 0:DIM],
                                                   rhs=o_mm[:, j, :],
                                                   start=(k == 0),
                                                   stop=(k == T_R - 1))
                              ng = (T_R + 3) // 4
                              gi = g0 // 4
                              if gl == 1:
                                  osum = o_mm[:, 0, :]
                              else:
                                  ost = op.tile([P, P], F32, tag=f"os{ri}")
                                  nc.vector.tensor_reduce(
                                      out=ost[:],
                                      in_=o_mm[:, 0:gl, :].rearrange(
                                          "p g d -> p d g"),
                                      axis=mybir.AxisListType.X,
                                      op=mybir.AluOpType.add)
                                  osum = ost[:]
                              nc.tensor.matmul(dens[:, ri * P:(ri + 1) * P],
                                               lhsT=ones_col[:], rhs=osum,
                                               start=(gi == 0),
                                               stop=(gi == ng - 1))

                      # ---- normalize + combine
                      if mode != "full":
                          ot = ep.tile([P, DIM], F32, tag="out_t")
                          if mode == "gather":
                              nc.vector.memset(ot[:], 0.0)
                          else:
                              nc.vector.tensor_copy(ot[:], num_p[:])
                          nc.sync.dma_start(out_d[rows, :], ot[:])
                          continue
                      dens_s = ep.tile([1, 3 * P], F32, tag="dens_s")
                      nc.vector.tensor_copy(dens_s[:], dens[:])
                      dcol = tps.tile([P, 3], F32, tag="dcol")
                      for ri in range(3):
                          nc.tensor.transpose(dcol[:, ri:ri + 1],
                                              dens_s[:, ri * P:(ri + 1) * P],
                                              ident[:1, :1])
                      r_p = ep.tile([P, 1], F32, tag="r_p")
                      nc.vector.tensor_scalar(out=r_p[:], in0=dcol[:, 0:1],
                                              scalar1=3.0, scalar2=None,
                                              op0=mybir.AluOpType.mult)
                      nc.vector.reciprocal(r_p[:], r_p[:])
                      r_c = ep.tile([P, 1], F32, tag="r_c")
                      nc.vector.tensor_scalar(out=r_c[:], in0=dcol[:, 1:2],
                                              scalar1=3.0, scalar2=None,
                                              op0=mybir.AluOpType.mult)
                      nc.vector.reciprocal(r_c[:], r_c[:])
                      r_s = ep.tile([P, 1], F32, tag="r_s")
                      nc.vector.tensor_scalar(out=r_s[:], in0=dcol[:, 2:3],
                                              scalar1=1.0, scalar2=3.0,
                                              op0=mybir.AluOpType.max,
                                              op1=mybir.AluOpType.mult)
                      nc.vector.reciprocal(r_s[:], r_s[:])

                      acc = ep.tile([P, DIM], F32, tag="acc")
                      numT = ep.tile([DIM, P], F32, tag="numT")
                      nc.scalar.copy(numT[:], num_p[:])
                      numn = tps.tile([P, DIM], F32, tag="numn")
                      nc.tensor.transpose(numn[:], numT[:], ident[:])
                      nc.vector.tensor_scalar(out=acc[:], in0=numn[:],
                                              scalar1=r_p[:], scalar2=None,
                                              op0=mybir.AluOpType.mult)
                      numT2 = ep.tile([DIM, P], F32, tag="numT2")
                      nc.scalar.copy(numT2[:], num_c[:])
                      numn2 = tps.tile([P, DIM], F32, tag="numn")
                      nc.tensor.transpose(numn2[:], numT2[:], ident[:])
                      tmp = ep.tile([P, DIM], F32, tag="tmp")
                      nc.vector.tensor_scalar(out=tmp[:], in0=numn2[:],
                                              scalar1=r_c[:], scalar2=None,
                                              op0=mybir.AluOpType.mult)
                      nc.vector.tensor_add(acc[:], acc[:], tmp[:])
                      # SAGE: (num_s @ W_l) scaled
                      numT3 = ep.tile([DIM, P], F32, tag="numT3")
                      nc.scalar.copy(numT3[:], num_s[:])
                      z_ps = tps.tile([DIM, P], F32, tag="numn")
                      nc.tensor.matmul(z_ps[:], lhsT=wl_t[:], rhs=numT3[:],
                                       start=True, stop=True)
                      z_s = ep.tile([DIM, P], F32, tag="z_s")
                      nc.scalar.copy(z_s[:], z_ps[:])
                      zn = tps.tile([P, DIM], F32, tag="numn")
                      nc.tensor.transpose(zn[:], z_s[:], ident[:])
                      tmp2 = ep.tile([P, DIM], F32, tag="tmp2")
                      nc.vector.tensor_scalar(out=tmp2[:], in0=zn[:],
                                              scalar1=r_s[:], scalar2=None,
                                              op0=mybir.AluOpType.mult)
                      nc.vector.tensor_add(acc[:], acc[:], tmp2[:])
                      xr_t = ep.tile([P, DIM], F32, tag="xr_t")
                      nc.sync.dma_start(xr_t[:], xr3_l[rows, :])
                      nc.vector.tensor_add(acc[:], acc[:], xr_t[:])
                      out_t = ep.tile([P, DIM], F32, tag="out_t")
                      nc.vector.tensor_add(out_t[:], acc[:], bias_bc[:])
                      nc.sync.dma_start(out_d[rows, :], out_t[:])

    nc.compile()
    return nc


# ---------------------------------------------------------------- entry

def _run(inputs, trace=False):
    s = np.asarray(inputs["s"])
    edge_index = np.asarray(inputs["edge_index"])
    x = np.asarray(inputs["embed_weight"], dtype=np.float32)
    W_p = np.asarray(inputs["W_p"], np.float32)
    W_c = np.asarray(inputs["W_c"], np.float32)
    W_l = np.asarray(inputs["W_l"], np.float32)
    W_r = np.asarray(inputs["W_r"], np.float32)
    att = np.stack([np.asarray(inputs["att_src_p"], np.float32),
                    np.asarray(inputs["att_dst_p"], np.float32),
                    np.asarray(inputs["att_src_c"], np.float32),
                    np.asarray(inputs["att_dst_c"], np.float32)], axis=1)
    bias = ((np.asarray(inputs["b_p"], np.float32)
             + np.asarray(inputs["b_c"], np.float32)
             + np.asarray(inputs["b_l"], np.float32)
             + np.asarray(inputs["b_r"], np.float32)) / 3.0)[None, :]

    uniq, inv = np.unique(s, return_inverse=True)
    U = len(uniq)
    G = N_T * U                               # real slots
    NBLK = -(-G // P)                         # total 128-slot blocks
    CB = -(-NBLK // CORES)                    # blocks per core
    NBLK = CB * CORES                         # padded to core multiple
    _CB[0] = CB
    slot_of = np.full(N_NODES, -1, np.int64)
    slot_of[uniq] = np.arange(U)

    T_G = max(_count_tiles(edge_index, r, True, slot_of, U, NBLK)
              for r in (0, 1))
    T_S = _count_tiles(edge_index, 2, False, slot_of, U, NBLK)

    rel_data = {}
    for name, r, loop, T in (("p", 0, True, T_G), ("c", 1, True, T_G),
                             ("s", 2, False, T_S)):
        sp, dp, dl, _ = _prep_rel(edge_index, r, loop, uniq, slot_of,
                                  U, NBLK, T)
        rel_data[name] = (sp, dp, dl, T)

    x_pad = np.zeros((NPAD, DIM), np.float32)
    x_pad[:N_NODES] = x
    x_t = np.ascontiguousarray(x_pad.T)
    xr3 = x_pad @ (W_r / 3.0)

    # per-slot dst node ids (PADROW for padding slots)
    slots = np.arange(NBLK * P)
    valid = slots < G
    node_of_slot = np.where(valid, uniq[np.minimum(slots % U, U - 1)], PADROW)
    node_of_slot[~valid] = PADROW
    xr3_slots = xr3[node_of_slot].astype(np.float32)        # [NBLK*P, DIM]

    nc = build_program(T_G, T_S)

    in_maps = []
    for core in range(CORES):
        bsl = slice(core * CB, (core + 1) * CB)
        m = {
            "x_pad": x_pad, "x_t": x_t, "w_p": W_p, "w_c": W_c, "w_l": W_l,
            "att": att, "bias": bias,
            "xr3_l": xr3_slots[core * CB * P:(core + 1) * CB * P],
        }
        for name in ("p", "c", "s"):
            sp, dp, dl, T = rel_data[name]
            S = T * 8
            m[f"sidx_{name}"] = _idx_image(sp[bsl]).reshape(-1, S)
            if name != "s":
                m[f"didx_{name}"] = _idx_image(dp[bsl]).reshape(-1, S)
            m[f"dloc_{name}"] = _dloc_image(dl[bsl], T).reshape(-1, T)
        in_maps.append(m)

    res = bass_utils.run_bass_kernel_spmd(nc, in_maps,
                                          core_ids=list(range(CORES)),
                                          trace=trace)
    full = np.concatenate([res.results[c]["out"] for c in range(CORES)],
                          axis=0)                           # [NBLK*P, DIM]
    sel = (np.arange(N_T)[:, None] * U + inv[None, :])      # [N_T, |s|]
    return full[sel], res


def kernel(**inputs):
    """Full (unsharded) inputs -> full output [N_T, len(s), DIM] float32."""
    out, _ = _run(inputs)
    return out
